# revision 39
# baseline (speedup 1.0000x reference)
"""Trainium2 Bass kernel for the attention-pooling layer:

    h    = tanh(x @ W^T + w_bias + b_p)   # [B, S, H]
    s    = h @ q                          # [B, S]
    attn = softmax(s, axis=1)[..., None]  # [B, S, 1]
    f_p  = sum(attn * x, axis=1)          # [B, H]

Sharding: data-parallel over batch across 8 NeuronCores (4 samples per
core); W / biases / q replicated. No cross-core communication needed.

Per-core dataflow (per sample):
  - host supplies x pre-transposed (xT[k, m]) so the contraction dim is on
    SBUF partitions; W supplied as W^T (wT[k, o]).
  - PE computes h^T tiles [o=128, m=512] accumulating k in PSUM (float32r:
    full fp32 storage, 1 cycle/row on the PE at N>=256).
  - ScalarE applies tanh(h + bias) straight out of PSUM (bias is
    per-partition in the h^T orientation).
  - PE contracts tanh(h)^T with q (per o-block) into scores s [1, 512].
  - softmax over the sample's 2048 scores on ACT/DVE (partition 0).
  - GpSimd broadcasts attn across partitions; VectorE fused
    multiply+reduce (tensor_tensor_reduce) against freshly re-loaded xT
    tiles produces f_p^T [128, 1] per h-block.
"""

import numpy as np

B, S, H = 32, 2048, 1024
N_CORES = 8
B_PC = B // N_CORES  # samples per core

P = 128          # partitions
N_CH = 512       # matmul moving-operand chunk (fp32 PSUM bank limit)
XT_CH = 1024     # xT load-tile free size (DMA batching granularity)

MM_DTYPE = "float32r"  # "float32r" (fast, ~tf32 matmul precision) or "float32"
STAGE = "full"  # debug knob: "mm" stops after scores; "nosm" skips softmax math

_BUILD_CACHE = {}


def _build(bpc, s, h, n_ch, xt_ch, mm_dtype, stage="full"):
    """Build + compile the per-core Bass program. Returns the Bacc module."""
    from contextlib import ExitStack

    import concourse.bass as bass  # noqa: F401
    import concourse.tile as tile
    from concourse import bacc, mybir

    f32 = mybir.dt.float32
    mm_dt = getattr(mybir.dt, mm_dtype)
    AF = mybir.ActivationFunctionType
    ALU = mybir.AluOpType
    AX = mybir.AxisListType

    m_tot = bpc * s
    kb_n = h // P          # k blocks (also h blocks for pooling)
    ob_n = h // P          # o blocks
    g_n = s // xt_ch       # xT load groups per sample
    mc_n = xt_ch // n_ch   # matmul chunks per load group

    nc = bacc.Bacc("TRN2", target_bir_lowering=False, debug=False,
                   num_devices=N_CORES)

    xT = nc.dram_tensor("xT", [h, m_tot], f32, kind="ExternalInput").ap()
    xn = nc.dram_tensor("xn", [m_tot, h], f32, kind="ExternalInput").ap()
    wT = nc.dram_tensor("wT", [h, h], f32, kind="ExternalInput").ap()
    bias = nc.dram_tensor("bias", [h], f32, kind="ExternalInput").ap()
    q = nc.dram_tensor("q", [h], f32, kind="ExternalInput").ap()
    f_p = nc.dram_tensor("f_p", [bpc, h], f32, kind="ExternalOutput").ap()
    attn = nc.dram_tensor("attn", [bpc, s], f32, kind="ExternalOutput").ap()

    def as_mm(ap):
        """View an fp32 AP as the matmul dtype (same 4-byte storage)."""
        return ap if mm_dt == f32 else ap.bitcast(mm_dt)

    with tile.TileContext(nc) as tc, ExitStack() as ctx:
        wt_pool = ctx.enter_context(tc.tile_pool(name="wt_pool", bufs=kb_n))
        const_pool = ctx.enter_context(tc.tile_pool(name="const_pool", bufs=1))
        xt_pool = ctx.enter_context(
            tc.tile_pool(name="xt_pool", bufs=2 * kb_n))
        th_pool = ctx.enter_context(tc.tile_pool(name="th_pool", bufs=4))
        xp_pool = ctx.enter_context(tc.tile_pool(name="xp_pool", bufs=3))
        sm_pool = ctx.enter_context(tc.tile_pool(name="sm_pool", bufs=2))
        one_pool = ctx.enter_context(tc.tile_pool(name="one_pool", bufs=1))
        xn_pool = ctx.enter_context(tc.tile_pool(name="xn_pool", bufs=12))
        hps_pool = ctx.enter_context(
            tc.tile_pool(name="hps_pool", bufs=2, space="PSUM"))
        sps_pool = ctx.enter_context(
            tc.tile_pool(name="sps_pool", bufs=2, space="PSUM"))
        bcps_pool = ctx.enter_context(
            tc.tile_pool(name="bcps_pool", bufs=2, space="PSUM"))

        # --- resident constants ---
        wt_tiles = []
        for kb in range(kb_n):
            w_t = wt_pool.tile([P, h], mm_dt, name=f"w_{kb}", tag="wt")
            nc.sync.dma_start(w_t[:], as_mm(wT[kb * P:(kb + 1) * P, :]))
            wt_tiles.append(w_t)
        # q / bias laid out [128, ob_n]: column j = block j, partition = offset
        q_sb = const_pool.tile([P, ob_n], mm_dt, name="q_sb", tag="q_sb")
        nc.sync.dma_start(q_sb[:], as_mm(q.rearrange("(j p) -> p j", p=P)))
        bias_sb = const_pool.tile([P, ob_n], f32, name="bias_sb", tag="bias_sb")
        nc.sync.dma_start(bias_sb[:], bias.rearrange("(j p) -> p j", p=P))
        ones_sb = const_pool.tile([1, P], f32, name="ones_sb", tag="ones_sb")
        nc.vector.memset(ones_sb[:], 1.0)

        for b in range(bpc):
            s_sb = sm_pool.tile([1, s], f32, name="s_sb", tag="s_sb")
            xn_tiles = []
            if stage == "pepool":
                # natural-layout x tiles for PE pooling (loads don't depend
                # on attn, so they overlap the main matmul phase)
                for mb in range(s // P):
                    xn_t = xn_pool.tile([P, h], mm_dt, name="xn_t", tag="xn")
                    nc.sync.dma_start(
                        xn_t[:],
                        as_mm(xn[b * s + mb * P: b * s + (mb + 1) * P, :]))
                    xn_tiles.append(xn_t)
            for g in range(g_n):
                xts = []
                for kb in range(kb_n):
                    x_t = xt_pool.tile([P, xt_ch], mm_dt, name="x_t", tag="xt")
                    nc.sync.dma_start(
                        x_t[:],
                        as_mm(xT[kb * P:(kb + 1) * P,
                                 b * s + g * xt_ch: b * s + (g + 1) * xt_ch]))
                    xts.append(x_t)
                # one scores-PSUM tile per mc chunk (separate accumulation
                # regions; all stay pending across the ob loop)
                s_pss = [sps_pool.tile([1, n_ch], f32, name=f"s_ps{mc}",
                                       tag="s_ps", bufs=mc_n)
                         for mc in range(mc_n)]
                for ob in range(ob_n):
                    # one h^T tile spans the whole group (xt_ch wide) so the
                    # tanh runs as a single large ACT op (amortizes the
                    # per-op ScalarE overhead, which dominated ACT busy time)
                    h_ps = hps_pool.tile([P, xt_ch], f32, name="h_ps",
                                         tag="h_ps")
                    for mc in range(mc_n):
                        msl = slice(mc * n_ch, (mc + 1) * n_ch)
                        for kb in range(kb_n):
                            nc.tensor.matmul(
                                h_ps[:, msl],
                                lhsT=wt_tiles[kb][:, ob * P:(ob + 1) * P],
                                rhs=xts[kb][:, msl],
                                start=(kb == 0), stop=(kb == kb_n - 1),
                            )
                    th = th_pool.tile([P, xt_ch], mm_dt, name="th", tag="th")
                    nc.scalar.activation(th[:], h_ps[:], AF.Tanh,
                                         bias=bias_sb[:, ob:ob + 1])
                    for mc in range(mc_n):
                        msl = slice(mc * n_ch, (mc + 1) * n_ch)
                        nc.tensor.matmul(
                            s_pss[mc][:],
                            lhsT=q_sb[:, ob:ob + 1],
                            rhs=th[:, msl],
                            start=(ob == 0), stop=(ob == ob_n - 1),
                        )
                for mc in range(mc_n):
                    nc.vector.tensor_copy(
                        s_sb[:, g * xt_ch + mc * n_ch:
                             g * xt_ch + (mc + 1) * n_ch], s_pss[mc][:])
            if stage == "mm":
                nc.sync.dma_start(attn[b:b + 1, :], s_sb[:])
                fp_sb = one_pool.tile([P, kb_n], f32, name="fp_sb",
                                      tag="fp_sb", bufs=2)
                nc.vector.memset(fp_sb[:], 0.0)
                nc.sync.dma_start(f_p[b].rearrange("(j p) -> p j", p=P),
                                  fp_sb[:])
                continue
            # --- softmax over this sample's s ---
            rmax = one_pool.tile([1, 1], f32, name="rmax", tag="rmax", bufs=2)
            nc.vector.tensor_reduce(rmax[:], s_sb[:], axis=AX.X, op=ALU.max)
            negmax = one_pool.tile([1, 1], f32, name="negmax", tag="negmax",
                                   bufs=2)
            nc.vector.tensor_scalar_mul(negmax[:], rmax[:], -1.0)
            p_sb = sm_pool.tile([1, s], f32, name="p_sb", tag="p_sb", bufs=1)
            ssum = one_pool.tile([1, 1], f32, name="ssum", tag="ssum", bufs=2)
            nc.scalar.activation(p_sb[:], s_sb[:], AF.Exp, bias=negmax[:],
                                 accum_out=ssum[:])
            rinv = one_pool.tile([1, 1], f32, name="rinv", tag="rinv", bufs=2)
            nc.vector.reciprocal(rinv[:], ssum[:])
            attn_sb = sm_pool.tile([1, s], f32, name="attn_sb", tag="attn_sb",
                                   bufs=1)
            nc.vector.tensor_scalar_mul(attn_sb[:], p_sb[:], rinv[:])
            if stage == "nosm":
                nc.sync.dma_start(attn[b:b + 1, :], attn_sb[:])
                fp_sb = one_pool.tile([P, kb_n], f32, name="fp_sb",
                                      tag="fp_sb", bufs=2)
                nc.vector.memset(fp_sb[:], 0.0)
                nc.sync.dma_start(f_p[b].rearrange("(j p) -> p j", p=P),
                                  fp_sb[:])
                continue
            if stage == "full":
                # DVE pooling against broadcast attn (HW-proven path)
                nc.sync.dma_start(attn[b:b + 1, :], attn_sb[:])
                attn_bc = sm_pool.tile([P, s], f32, name="attn_bc",
                                       tag="attn_bc", bufs=1)
                for j in range(s // n_ch):
                    jsl = slice(j * n_ch, (j + 1) * n_ch)
                    bc_ps = bcps_pool.tile([P, n_ch], f32, name="bc_ps",
                                           tag="bc_ps")
                    nc.tensor.matmul(bc_ps[:], lhsT=ones_sb[:],
                                     rhs=attn_sb[:, jsl], start=True,
                                     stop=True)
                    nc.vector.tensor_copy(attn_bc[:, jsl], bc_ps[:])
                fp_sb = one_pool.tile([P, kb_n], f32, name="fp_sb",
                                      tag="fp_sb", bufs=2)
                for hb in range(kb_n):
                    xp_t = xp_pool.tile([P, s], f32, name="xp_t", tag="xp")
                    nc.sync.dma_start(
                        xp_t[:], xT[hb * P:(hb + 1) * P, b * s:(b + 1) * s])
                    scr = sm_pool.tile([P, s], f32, name="scr", tag="scr",
                                       bufs=1)
                    nc.vector.tensor_mul(scr[:], xp_t[:], attn_bc[:])
                    nc.vector.tensor_reduce(fp_sb[:, hb:hb + 1], scr[:],
                                            axis=AX.X, op=ALU.add)
                nc.sync.dma_start(f_p[b].rearrange("(j p) -> p j", p=P),
                                  fp_sb[:])
                continue
            # --- PE pooling (last sample in "full", or stage "pepool"):
            # attn crosses partitions via 16 PE-transposes of [1,128] chunks,
            # then f_p[b] = attn^T @ x contracts tokens on the PE. This keeps
            # the end-of-kernel tail off the (slower) DVE path.
            nc.sync.dma_start(attn[b:b + 1, :], attn_sb[:])
            attnT_ps = bcps_pool.tile([P, s // P], f32, name="attnT_ps",
                                      tag="attnT_ps", bufs=1)
            for mb in range(s // P):
                nc.tensor.transpose(attnT_ps[:, mb:mb + 1],
                                    attn_sb[:, mb * P:(mb + 1) * P],
                                    ones_sb[:, 0:1])
            attnT = sm_pool.tile([P, s // P], mm_dt, name="attnT", tag="attnT",
                                 bufs=2)
            nc.vector.tensor_copy(attnT[:], attnT_ps[:])
            fp_sb = sm_pool.tile([1, h], f32, name="fp_sb", tag="fp_sb",
                                 bufs=2)
            n_half = h // n_ch
            fp_pss = [sps_pool.tile([1, n_ch], f32, name=f"fp_ps{half}",
                                    tag="s_ps")
                      for half in range(n_half)]
            for mb in range(s // P):
                for half in range(n_half):
                    hsl = slice(half * n_ch, (half + 1) * n_ch)
                    nc.tensor.matmul(
                        fp_pss[half][:],
                        lhsT=attnT[:, mb:mb + 1],
                        rhs=xn_tiles[mb][:, hsl],
                        start=(mb == 0), stop=(mb == s // P - 1),
                    )
            for half in range(n_half):
                hsl = slice(half * n_ch, (half + 1) * n_ch)
                nc.vector.tensor_copy(fp_sb[:, hsl], fp_pss[half][:])
            nc.sync.dma_start(f_p[b:b + 1, :], fp_sb[:])

    nc.compile()
    return nc


def _get_nc():
    key = (B_PC, S, H, N_CH, XT_CH, MM_DTYPE, STAGE)
    if key not in _BUILD_CACHE:
        _BUILD_CACHE[key] = _build(*key)
    return _BUILD_CACHE[key]


def kernel(x, W, w_bias, b_p, q):
    from concourse.bass_utils import run_bass_kernel_spmd

    x = np.asarray(x, dtype=np.float32)
    W = np.asarray(W, dtype=np.float32)
    bias = (np.asarray(w_bias, dtype=np.float32)
            + np.asarray(b_p, dtype=np.float32))
    q = np.asarray(q, dtype=np.float32)
    wT = np.ascontiguousarray(W.T)

    in_maps = []
    for c in range(N_CORES):
        xc = x[c * B_PC:(c + 1) * B_PC].reshape(B_PC * S, H)
        in_maps.append({
            "xT": np.ascontiguousarray(xc.T),
            "xn": xc,
            "wT": wT,
            "bias": bias,
            "q": q,
        })

    nc = _get_nc()
    res = run_bass_kernel_spmd(nc, in_maps, list(range(N_CORES)))
    f_p = np.concatenate([res.results[c]["f_p"] for c in range(N_CORES)], 0)
    attn = np.concatenate([res.results[c]["attn"] for c in range(N_CORES)], 0)
    return f_p, attn[..., None]


# revision 43
# speedup vs baseline: 1.0226x; 1.0226x over previous
"""Trainium2 Bass kernel for the attention-pooling layer:

    h    = tanh(x @ W^T + w_bias + b_p)   # [B, S, H]
    s    = h @ q                          # [B, S]
    attn = softmax(s, axis=1)[..., None]  # [B, S, 1]
    f_p  = sum(attn * x, axis=1)          # [B, H]

Sharding: data-parallel over batch across 8 NeuronCores (4 samples per
core); W / biases / q replicated. No cross-core communication needed.

Per-core dataflow (per sample):
  - host supplies x pre-transposed (xT[k, m]) so the contraction dim is on
    SBUF partitions; W supplied as W^T (wT[k, o]).
  - PE computes h^T tiles [o=128, m=512] accumulating k in PSUM (float32r:
    full fp32 storage, 1 cycle/row on the PE at N>=256).
  - ScalarE applies tanh(h + bias) straight out of PSUM (bias is
    per-partition in the h^T orientation).
  - PE contracts tanh(h)^T with q (per o-block) into scores s [1, 512].
  - softmax over the sample's 2048 scores on ACT/DVE (partition 0).
  - GpSimd broadcasts attn across partitions; VectorE fused
    multiply+reduce (tensor_tensor_reduce) against freshly re-loaded xT
    tiles produces f_p^T [128, 1] per h-block.
"""

import numpy as np

B, S, H = 32, 2048, 1024
N_CORES = 8
B_PC = B // N_CORES  # samples per core

P = 128          # partitions
N_CH = 512       # matmul moving-operand chunk (fp32 PSUM bank limit)
XT_CH = 1024     # xT load-tile free size (DMA batching granularity)

MM_DTYPE = "float32r"  # "float32r" (fast, ~tf32 matmul precision) or "float32"
STAGE = "full"  # debug knob: "mm" stops after scores; "nosm" skips softmax math

_BUILD_CACHE = {}


def _build(bpc, s, h, n_ch, xt_ch, mm_dtype, stage="full"):
    """Build + compile the per-core Bass program. Returns the Bacc module."""
    from contextlib import ExitStack

    import concourse.bass as bass  # noqa: F401
    import concourse.tile as tile
    from concourse import bacc, mybir

    f32 = mybir.dt.float32
    mm_dt = getattr(mybir.dt, mm_dtype)
    AF = mybir.ActivationFunctionType
    ALU = mybir.AluOpType
    AX = mybir.AxisListType

    m_tot = bpc * s
    kb_n = h // P          # k blocks (also h blocks for pooling)
    ob_n = h // P          # o blocks
    g_n = s // xt_ch       # xT load groups per sample
    mc_n = xt_ch // n_ch   # matmul chunks per load group

    nc = bacc.Bacc("TRN2", target_bir_lowering=False, debug=False,
                   num_devices=N_CORES)

    xT = nc.dram_tensor("xT", [h, m_tot], f32, kind="ExternalInput").ap()
    xn = nc.dram_tensor("xn", [m_tot, h], f32, kind="ExternalInput").ap()
    wT = nc.dram_tensor("wT", [h, h], f32, kind="ExternalInput").ap()
    bias = nc.dram_tensor("bias", [h], f32, kind="ExternalInput").ap()
    q = nc.dram_tensor("q", [h], f32, kind="ExternalInput").ap()
    f_p = nc.dram_tensor("f_p", [bpc, h], f32, kind="ExternalOutput").ap()
    attn = nc.dram_tensor("attn", [bpc, s], f32, kind="ExternalOutput").ap()

    def as_mm(ap):
        """View an fp32 AP as the matmul dtype (same 4-byte storage)."""
        return ap if mm_dt == f32 else ap.bitcast(mm_dt)

    with tile.TileContext(nc) as tc, ExitStack() as ctx:
        wt_pool = ctx.enter_context(tc.tile_pool(name="wt_pool", bufs=kb_n))
        const_pool = ctx.enter_context(tc.tile_pool(name="const_pool", bufs=1))
        xt_pool = ctx.enter_context(
            tc.tile_pool(name="xt_pool", bufs=2 * kb_n))
        th_pool = ctx.enter_context(tc.tile_pool(name="th_pool", bufs=4))
        xp_pool = ctx.enter_context(tc.tile_pool(name="xp_pool", bufs=3))
        sm_pool = ctx.enter_context(tc.tile_pool(name="sm_pool", bufs=2))
        one_pool = ctx.enter_context(tc.tile_pool(name="one_pool", bufs=1))
        xn_pool = ctx.enter_context(tc.tile_pool(name="xn_pool", bufs=12))
        hps_pool = ctx.enter_context(
            tc.tile_pool(name="hps_pool", bufs=2, space="PSUM"))
        sps_pool = ctx.enter_context(
            tc.tile_pool(name="sps_pool", bufs=2, space="PSUM"))
        bcps_pool = ctx.enter_context(
            tc.tile_pool(name="bcps_pool", bufs=2, space="PSUM"))

        # --- resident constants ---
        wt_tiles = []
        for kb in range(kb_n):
            w_t = wt_pool.tile([P, h], mm_dt, name=f"w_{kb}", tag="wt")
            nc.sync.dma_start(w_t[:], as_mm(wT[kb * P:(kb + 1) * P, :]))
            wt_tiles.append(w_t)
        # q / bias laid out [128, ob_n]: column j = block j, partition = offset
        q_sb = const_pool.tile([P, ob_n], mm_dt, name="q_sb", tag="q_sb")
        nc.sync.dma_start(q_sb[:], as_mm(q.rearrange("(j p) -> p j", p=P)))
        bias_sb = const_pool.tile([P, ob_n], f32, name="bias_sb", tag="bias_sb")
        nc.sync.dma_start(bias_sb[:], bias.rearrange("(j p) -> p j", p=P))
        ones_f = const_pool.tile([1, P], f32, name="ones_f", tag="ones_f")
        nc.vector.memset(ones_f[:], 1.0)
        ones_sb = const_pool.tile([1, P], mm_dt, name="ones_sb",
                                  tag="ones_sb")
        nc.vector.tensor_copy(ones_sb[:], ones_f[:])

        for b in range(bpc):
            s_sb = sm_pool.tile([1, s], f32, name="s_sb", tag="s_sb")
            xn_tiles = []
            if stage == "pepool":
                # natural-layout x tiles for PE pooling (loads don't depend
                # on attn, so they overlap the main matmul phase)
                for mb in range(s // P):
                    xn_t = xn_pool.tile([P, h], mm_dt, name="xn_t", tag="xn")
                    nc.sync.dma_start(
                        xn_t[:],
                        as_mm(xn[b * s + mb * P: b * s + (mb + 1) * P, :]))
                    xn_tiles.append(xn_t)
            # Load-group spans: the very first group of the kernel is split
            # into n_ch-wide slivers so the PE starts as soon as the first
            # small DMAs land instead of waiting for a full xt_ch load.
            if b == 0:
                spans = [(i * n_ch, n_ch) for i in range(mc_n)]
                spans += [(g * xt_ch, xt_ch) for g in range(1, g_n)]
            else:
                spans = [(g * xt_ch, xt_ch) for g in range(g_n)]
            for (m0, gw) in spans:
                gw_mc = gw // n_ch
                xts = []
                for kb in range(kb_n):
                    x_t = xt_pool.tile([P, gw], mm_dt, name="x_t", tag="xt",
                                       padded_shape=[P, xt_ch])
                    nc.sync.dma_start(
                        x_t[:],
                        as_mm(xT[kb * P:(kb + 1) * P,
                                 b * s + m0: b * s + m0 + gw]))
                    xts.append(x_t)
                # one scores-PSUM tile per mc chunk (separate accumulation
                # regions; all stay pending across the ob loop)
                s_pss = [sps_pool.tile([1, n_ch], f32, name=f"s_ps{mc}",
                                       tag="s_ps", bufs=mc_n)
                         for mc in range(gw_mc)]
                for ob in range(ob_n):
                    # one h^T tile spans the whole group so the tanh runs as
                    # a single large ACT op (amortizes the per-op ScalarE
                    # overhead, which dominated ACT busy time)
                    h_ps = hps_pool.tile([P, gw], f32, name="h_ps",
                                         tag="h_ps", padded_shape=[P, xt_ch])
                    for mc in range(gw_mc):
                        msl = slice(mc * n_ch, (mc + 1) * n_ch)
                        for kb in range(kb_n):
                            nc.tensor.matmul(
                                h_ps[:, msl],
                                lhsT=wt_tiles[kb][:, ob * P:(ob + 1) * P],
                                rhs=xts[kb][:, msl],
                                start=(kb == 0), stop=(kb == kb_n - 1),
                            )
                    th = th_pool.tile([P, gw], mm_dt, name="th", tag="th",
                                      padded_shape=[P, xt_ch])
                    nc.scalar.activation(th[:], h_ps[:], AF.Tanh,
                                         bias=bias_sb[:, ob:ob + 1])
                    for mc in range(gw_mc):
                        msl = slice(mc * n_ch, (mc + 1) * n_ch)
                        nc.tensor.matmul(
                            s_pss[mc][:],
                            lhsT=q_sb[:, ob:ob + 1],
                            rhs=th[:, msl],
                            start=(ob == 0), stop=(ob == ob_n - 1),
                        )
                for mc in range(gw_mc):
                    nc.vector.tensor_copy(
                        s_sb[:, m0 + mc * n_ch: m0 + (mc + 1) * n_ch],
                        s_pss[mc][:])
            if stage == "mm":
                nc.sync.dma_start(attn[b:b + 1, :], s_sb[:])
                fp_sb = one_pool.tile([P, kb_n], f32, name="fp_sb",
                                      tag="fp_sb", bufs=2)
                nc.vector.memset(fp_sb[:], 0.0)
                nc.sync.dma_start(f_p[b].rearrange("(j p) -> p j", p=P),
                                  fp_sb[:])
                continue
            # --- softmax over this sample's s ---
            rmax = one_pool.tile([1, 1], f32, name="rmax", tag="rmax", bufs=2)
            nc.vector.tensor_reduce(rmax[:], s_sb[:], axis=AX.X, op=ALU.max)
            negmax = one_pool.tile([1, 1], f32, name="negmax", tag="negmax",
                                   bufs=2)
            nc.vector.tensor_scalar_mul(negmax[:], rmax[:], -1.0)
            p_sb = sm_pool.tile([1, s], f32, name="p_sb", tag="p_sb", bufs=1)
            ssum = one_pool.tile([1, 1], f32, name="ssum", tag="ssum", bufs=2)
            nc.scalar.activation(p_sb[:], s_sb[:], AF.Exp, bias=negmax[:],
                                 accum_out=ssum[:])
            rinv = one_pool.tile([1, 1], f32, name="rinv", tag="rinv", bufs=2)
            nc.vector.reciprocal(rinv[:], ssum[:])
            attn_sb = sm_pool.tile([1, s], mm_dt, name="attn_sb",
                                   tag="attn_sb", bufs=1)
            nc.vector.tensor_scalar_mul(attn_sb[:], p_sb[:], rinv[:])
            if stage == "nosm":
                nc.sync.dma_start(attn[b:b + 1, :], attn_sb[:])
                fp_sb = one_pool.tile([P, kb_n], f32, name="fp_sb",
                                      tag="fp_sb", bufs=2)
                nc.vector.memset(fp_sb[:], 0.0)
                nc.sync.dma_start(f_p[b].rearrange("(j p) -> p j", p=P),
                                  fp_sb[:])
                continue
            if stage == "full":
                # DVE pooling against broadcast attn (HW-proven path)
                nc.sync.dma_start(attn[b:b + 1, :],
                                  attn_sb[:] if mm_dt == f32
                                  else attn_sb[:].bitcast(f32))
                attn_bc = sm_pool.tile([P, s], f32, name="attn_bc",
                                       tag="attn_bc", bufs=1)
                for j in range(s // n_ch):
                    jsl = slice(j * n_ch, (j + 1) * n_ch)
                    bc_ps = bcps_pool.tile([P, n_ch], f32, name="bc_ps",
                                           tag="bc_ps")
                    nc.tensor.matmul(bc_ps[:], lhsT=ones_sb[:],
                                     rhs=attn_sb[:, jsl], start=True,
                                     stop=True)
                    nc.vector.tensor_copy(attn_bc[:, jsl], bc_ps[:])
                fp_sb = one_pool.tile([P, kb_n], f32, name="fp_sb",
                                      tag="fp_sb", bufs=2)
                for hb in range(kb_n):
                    xp_t = xp_pool.tile([P, s], f32, name="xp_t", tag="xp")
                    nc.sync.dma_start(
                        xp_t[:], xT[hb * P:(hb + 1) * P, b * s:(b + 1) * s])
                    scr = sm_pool.tile([P, s], f32, name="scr", tag="scr",
                                       bufs=1)
                    nc.vector.tensor_mul(scr[:], xp_t[:], attn_bc[:])
                    nc.vector.tensor_reduce(fp_sb[:, hb:hb + 1], scr[:],
                                            axis=AX.X, op=ALU.add)
                nc.sync.dma_start(f_p[b].rearrange("(j p) -> p j", p=P),
                                  fp_sb[:])
                continue
            # --- PE pooling (last sample in "full", or stage "pepool"):
            # attn crosses partitions via 16 PE-transposes of [1,128] chunks,
            # then f_p[b] = attn^T @ x contracts tokens on the PE. This keeps
            # the end-of-kernel tail off the (slower) DVE path.
            nc.sync.dma_start(attn[b:b + 1, :], attn_sb[:])
            attnT_ps = bcps_pool.tile([P, s // P], f32, name="attnT_ps",
                                      tag="attnT_ps", bufs=1)
            for mb in range(s // P):
                nc.tensor.transpose(attnT_ps[:, mb:mb + 1],
                                    attn_sb[:, mb * P:(mb + 1) * P],
                                    ones_sb[:, 0:1])
            attnT = sm_pool.tile([P, s // P], mm_dt, name="attnT", tag="attnT",
                                 bufs=2)
            nc.vector.tensor_copy(attnT[:], attnT_ps[:])
            fp_sb = sm_pool.tile([1, h], f32, name="fp_sb", tag="fp_sb",
                                 bufs=2)
            n_half = h // n_ch
            fp_pss = [sps_pool.tile([1, n_ch], f32, name=f"fp_ps{half}",
                                    tag="s_ps")
                      for half in range(n_half)]
            for mb in range(s // P):
                for half in range(n_half):
                    hsl = slice(half * n_ch, (half + 1) * n_ch)
                    nc.tensor.matmul(
                        fp_pss[half][:],
                        lhsT=attnT[:, mb:mb + 1],
                        rhs=xn_tiles[mb][:, hsl],
                        start=(mb == 0), stop=(mb == s // P - 1),
                    )
            for half in range(n_half):
                hsl = slice(half * n_ch, (half + 1) * n_ch)
                nc.vector.tensor_copy(fp_sb[:, hsl], fp_pss[half][:])
            nc.sync.dma_start(f_p[b:b + 1, :], fp_sb[:])

    nc.compile()
    return nc


def _get_nc():
    key = (B_PC, S, H, N_CH, XT_CH, MM_DTYPE, STAGE)
    if key not in _BUILD_CACHE:
        _BUILD_CACHE[key] = _build(*key)
    return _BUILD_CACHE[key]


def kernel(x, W, w_bias, b_p, q):
    from concourse.bass_utils import run_bass_kernel_spmd

    x = np.asarray(x, dtype=np.float32)
    W = np.asarray(W, dtype=np.float32)
    bias = (np.asarray(w_bias, dtype=np.float32)
            + np.asarray(b_p, dtype=np.float32))
    q = np.asarray(q, dtype=np.float32)
    wT = np.ascontiguousarray(W.T)

    in_maps = []
    for c in range(N_CORES):
        xc = x[c * B_PC:(c + 1) * B_PC].reshape(B_PC * S, H)
        in_maps.append({
            "xT": np.ascontiguousarray(xc.T),
            "xn": xc,
            "wT": wT,
            "bias": bias,
            "q": q,
        })

    nc = _get_nc()
    res = run_bass_kernel_spmd(nc, in_maps, list(range(N_CORES)))
    f_p = np.concatenate([res.results[c]["f_p"] for c in range(N_CORES)], 0)
    attn = np.concatenate([res.results[c]["attn"] for c in range(N_CORES)], 0)
    return f_p, attn[..., None]
